# revision 47
# baseline (speedup 1.0000x reference)
"""BLOOM attention block (B=2, S=2048, D=2048, H=16) on 8 Trainium2 NeuronCores.

Sharding: core c handles batch b=c//4 and head group g=c%4; head slot j on
core (b,g) computes global head 4*j+g, so every slot sees a narrow ALiBi
slope band (slot 0 = steepest slopes on every core) and the per-slot tile
plan can exploit ALiBi decay uniformly across the SPMD cores.

Numerics (fp8 = e4m3, scale 256 on weights):
  - Q = x8 @ wq8 (fp8 DoubleRow), stored fp8.
  - K = (x_hi+x_lo) @ wk8 (fp8 DR, x error cancelled), stored as an fp8
    hi+lo pair; the scores matmul contracts both planes in one DoubleRow
    pass, so K enters at ~bf16 fidelity at half the bf16 cost.
  - V = (x_hi+x_lo) @ wv8, stored bf16.
  - scores^T[sk,sq] = (K_hi+K_lo) @ Q^T via one DR matmul; the analytic
    per-column shift -c[sq] (host cummax of alibi[+mask]) is injected by a
    rank-1 fp8 DR matmul (128 * c/128); its quantization error is constant
    per column and cancels exactly in the softmax normalization.
  - exp on ACT with bias=alibi col (fp32) and scale=INV_NORM; pt in bf16.
  - ctx^T = V^T @ P^T and column sums (ones matmul) in bf16; the qkv v-bias
    is folded host-side (bv @ W_dense^T added to the output) since
    sum(P)=1 after normalization.
  - dense: ctx stored fp8; W_dense^T stored as fp8 hi+lo pair; 4 DR
    matmuls per psum tile (heads paired for the hi plane and for the lo
    plane), output bf16.
  - The far-distance clean tiles of slot 0 (steep slopes) are skipped
    entirely: exp(score + a[k]-a[sq]) underflows bf16 to exactly 0 there.
"""

import math
import time

import numpy as np

import bass_rust
import concourse.bass as bass
import concourse.mybir as mybir
import concourse.tile as tile
from concourse import bass_utils

import ml_dtypes

BF16_NP = ml_dtypes.bfloat16
F8_NP = ml_dtypes.float8_e4m3

B, S, D, H = 2, 2048, 2048, 16
HD = D // H  # 128
INV_NORM = 1.0 / math.sqrt(HD)
NCORES = 8
HPC = 4  # head slots per core
SQT = 512  # sq tile width (free dim of transposed score tiles)
NQT = S // SQT  # 4
NKT = S // 128  # 16 sk tiles
NDT = D // 128  # 16 contraction tiles
FD32 = mybir.dt.float32
BF16 = mybir.dt.bfloat16
FP8 = mybir.dt.float8e4
DR = mybir.MatmulPerfMode.DoubleRow
NEG_BIG = -1.0e9
WSCALE = 256.0
SHIFT_ONES = 128.0
QJ_ORDER = [3, 2, 1, 0]
PSUM_QKV = 8
PSUM_SCPS = 3
PSUM_CTXPS = 2
PSUM_SMPS = 2
PSUM_DPS = 1
PT_BUFS = 4
WORK_BUFS = 4
QX2_BUFS = 2
FLUSH_N = 2
SM_ON_ACT = False
CTXT_BUFS = 2
OUTSB_BUFS = 4
# ALiBi-decay cutoff distance per head slot (slot j holds heads 4j..4j+3;
# the shallowest slope in slot j is head 4j+3). A dropped key at distance
# > dist has weight < e^{12-52} = e^-40 of the column's max term (scores
# span +-6), i.e. zero at fp32 accumulation precision.
SLOPE_BASE = 2.0 ** (-(2.0 ** -(math.log2(H) - 3)))
SLOT_SKIP_DIST = [52.0 / (SLOPE_BASE ** (4 * j + 4)) for j in range(HPC)]


def _split_multi_waits(nc):
    """This toolchain's walrus accepts at most ONE sync wait per instruction;
    Tile emits multi-wait instructions. Move extra waits onto preceding NOPs
    on the same engine (waits execute in stream order, so semantics hold)."""
    for fn in nc.m.functions:
        for bb in fn.blocks:
            insts = bb.instructions
            i = 0
            while i < len(insts):
                inst = insts[i]
                si = inst.sync_info
                if si is not None and len(si.on_wait) > 1:
                    waits = list(si.on_wait)
                    carriers = []
                    for k, w in enumerate(waits[:-1]):
                        nop = mybir.InstNoOp(name=f"{inst.name}_sw{k}", ins=[], outs=[])
                        nop.engine = inst.engine
                        nop.sync_info = bass_rust.SyncInfo(on_wait=[w], on_update=[])
                        nc.register_instruction(nop, overwrite=True)
                        carriers.append(nop)
                    inst.sync_info = bass_rust.SyncInfo(
                        on_wait=[waits[-1]], on_update=si.on_update
                    )
                    insts[i:i] = carriers
                    i += len(carriers)
                i += 1


def _tile_plan(mode, trim):
    """plan[slot][qj][ki] is None (skip) or (kind, off, end): valid sq
    columns [off, end) of the tile. 'pat' needs the post-exp triangle zero.
    In causal mode, columns beyond the slot's ALiBi cutoff distance are
    trimmed away (right cut) and fully-cut tiles are skipped."""
    plans = []
    for slot in range(HPC):
        plan = []
        for qj in range(NQT):
            row = []
            for ki in range(NKT):
                if mode == "none":
                    row.append(("clean", 0, SQT))
                elif mode == "data":
                    row.append(("data", 0, SQT))
                else:  # causal: keys sk <= queries sq
                    sk_lo, sk_hi = 128 * ki, 128 * ki + 127
                    sq0 = SQT * qj
                    if sk_lo > sq0 + SQT - 1:
                        row.append(None)
                        continue
                    off = max(0, sk_lo - sq0)
                    dist = SLOT_SKIP_DIST[slot] if trim else 10 * S
                    end = min(SQT, sk_hi + int(dist) - sq0 + 1)
                    if end <= off:
                        row.append(None)
                    elif sk_hi <= sq0:
                        row.append(("clean", 0, end))
                    else:
                        row.append(("pat", off, end))
            plan.append(row)
        plans.append(plan)
    return plans


def _shift_fold(mode, trim):
    """Slots whose shift fits entirely in the fp32 act bias: when the
    per-head cummax range is small (shallow ALiBi slopes), subtracting a
    per-head CONSTANT keeps exp args bounded, so the rank-1 shift matmul
    can be dropped (the residual per-column factor cancels in the
    normalization)."""
    if mode == "causal":
        # requires the known BLOOM slope band per slot
        return [False, False, trim, trim]
    if mode == "none":
        return [True] * HPC  # c is a per-head constant for any alibi
    return [False] * HPC


def _build_program(mode, trim):
    """mode in {'none', 'causal', 'data'}; returns the Bass module. trim
    enables the ALiBi-decay tile skips/cuts and per-slot shift folding,
    valid only for the standard BLOOM alibi."""
    plans = _tile_plan(mode, trim)
    fold = _shift_fold(mode, trim)

    nc = bass.Bass()
    xh = nc.dram_tensor("xh", [D, S], FP8, kind="ExternalInput")
    wqt = nc.dram_tensor("wqt", [D, HPC * HD], FP8, kind="ExternalInput")
    wkt = nc.dram_tensor("wkt", [D, HPC * HD], FP8, kind="ExternalInput")
    wvt = nc.dram_tensor("wvt", [D, 2 * HPC * HD], FP8, kind="ExternalInput")
    wdh = nc.dram_tensor("wdh", [HPC * HD, D], FP8, kind="ExternalInput")
    wdl = nc.dram_tensor("wdl", [HPC * HD, D], FP8, kind="ExternalInput")
    bqk = nc.dram_tensor("bqk", [128, 2 * HPC], FD32, kind="ExternalInput")
    alib = nc.dram_tensor("alib", [128, HPC * NKT], FD32, kind="ExternalInput")
    negc = nc.dram_tensor("negc", [1, HPC * S], FP8, kind="ExternalInput")
    shiftw = nc.dram_tensor("shiftw", [1, 2 * 128], FP8, kind="ExternalInput")
    onesp1 = nc.dram_tensor("onesp1", [128, 1], BF16, kind="ExternalInput")
    ones1b = nc.dram_tensor("ones1b", [1, 128], BF16, kind="ExternalInput")
    maskt = None
    if mode == "data":
        maskt = nc.dram_tensor("maskt", [S, S], FD32, kind="ExternalInput")
    outp = nc.dram_tensor("outp", [S, D], BF16, kind="ExternalOutput")

    with tile.TileContext(nc) as tc:
        with tc.tile_pool(name="persist", bufs=1) as persist:
            # ---- persistent SBUF tensors -------------------------------
            qt_sb = persist.tile([128, HPC + 1, S], FP8)  # +1 junk plane
            khl_sb = persist.tile([128, 2, HPC, S], FP8)  # K hi/lo planes
            v_sb = persist.tile([128, NKT, HPC * HD], BF16)  # V native
            wdh_sb = persist.tile([128, HPC, D], FP8)
            wdl_sb = persist.tile([128, HPC, D], FP8)
            xh_sb = persist.tile([128, NDT, S], FP8)  # resident x_hi
            bqk_sb = persist.tile([128, 2 * HPC], FD32)
            alib_sb = persist.tile([128, HPC * NKT], FD32)
            negc_sb = persist.tile([1, HPC * S], FP8)
            shiftw_sb = persist.tile([1, 2, 128], FP8)
            onesp1_sb = persist.tile([128, 1], BF16)
            ones1b_sb = persist.tile([1, 128], BF16)

            def load_consts():
                # issued AFTER the bulk x/w loads: tiny DMAs must not
                # head-of-line-block the HWDGE dispatch queue at startup
                nc.gpsimd.dma_start(out=bqk_sb, in_=bqk[:])
                nc.gpsimd.dma_start(out=alib_sb, in_=alib[:])
                nc.gpsimd.dma_start(out=negc_sb, in_=negc[:])
                nc.gpsimd.dma_start(
                    out=shiftw_sb, in_=shiftw.rearrange("p (k j) -> p k j", k=2)
                )
                nc.gpsimd.dma_start(out=onesp1_sb, in_=onesp1[:])
                nc.gpsimd.dma_start(out=ones1b_sb, in_=ones1b[:])
                # junk plane read by the slot-3 scores matmul must be finite
                # (0 * NaN = NaN on the PE); placed after the bulk loads so
                # it never delays the startup-critical DMAs
                nc.gpsimd.memset(qt_sb[:, HPC, :], 0.0)


            xh_r = xh.rearrange("(dt p) s -> p dt s", p=128)
            wqt_r = wqt.rearrange("(dt p) f -> p dt f", p=128)
            wkt_r = wkt.rearrange("(dt p) f -> p dt f", p=128)
            wvt_r = wvt.rearrange("(dt p) (k f) -> p dt k f", p=128, k=2)

            def dr_proj(ps, w_sb, x_tiles, hs, first_start=True):
                """psum += sum over x planes/dt pairs of w^T x (DoubleRow).
                Emission is chunk-major (dt groups of 4) so the matmuls
                consume the x DMA chunks in arrival order."""
                order = []
                for c4 in range(4):
                    for xp in x_tiles:
                        for dtp in (4 * c4, 4 * c4 + 2):
                            order.append((xp, dtp))
                for n, (xp, dtp) in enumerate(order):
                    nc.tensor.matmul(
                        ps,
                        w_sb[:, dtp : dtp + 2, hs],
                        xp[:, dtp : dtp + 2, :],
                        start=(n == 0 and first_start),
                        stop=(n == len(order) - 1),
                        perf_mode=DR,
                    )

            # ---- phase 1: K+V projection (weights split hi/lo fp8, so
            # each dt needs ONE DoubleRow pass; x_hi is the only x input)
            with tc.tile_pool(name="wqp", bufs=1) as wqp:
                wq_sb = wqp.tile([128, NDT, HPC * HD], FP8)
                with (
                    tc.tile_pool(name="qkvw", bufs=1) as qkvw,
                    tc.tile_pool(name="p1w", bufs=2) as p1w,
                    tc.tile_pool(name="qkvps", bufs=PSUM_QKV, space="PSUM") as qkvps,
                ):
                    wk_sb = qkvw.tile([128, NDT, HPC * HD], FP8)
                    wv_sb = qkvw.tile([128, NDT, 2, HPC * HD], FP8)
                    # fine-grained chunked loads so first matmuls start early
                    for c4 in range(4):
                        dsl = slice(c4 * 4, (c4 + 1) * 4)
                        nc.sync.dma_start(out=wk_sb[:, dsl, :], in_=wkt_r[:, dsl, :])
                        nc.gpsimd.dma_start(out=xh_sb[:, dsl, :], in_=xh_r[:, dsl, :])
                    load_consts()
                    for c2 in range(2):
                        dsl = slice(c2 * 8, (c2 + 1) * 8)
                        nc.sync.dma_start(
                            out=wv_sb[:, dsl, :, :], in_=wvt_r[:, dsl, :, :]
                        )
                    for c2 in range(2):
                        dsl = slice(c2 * 8, (c2 + 1) * 8)
                        nc.sync.dma_start(out=wq_sb[:, dsl, :], in_=wqt_r[:, dsl, :])
                    for c2 in range(2):
                        nc.sync.dma_start(
                            out=wdh_sb[:, c2 * 2 : c2 * 2 + 2, :],
                            in_=wdh.rearrange("(h p) o -> p h o", p=128)[
                                :, c2 * 2 : c2 * 2 + 2, :
                            ],
                        )
                        nc.sync.dma_start(
                            out=wdl_sb[:, c2 * 2 : c2 * 2 + 2, :],
                            in_=wdl.rearrange("(h p) o -> p h o", p=128)[
                                :, c2 * 2 : c2 * 2 + 2, :
                            ],
                        )

                    def k_evac(ps_k, h, ssl):
                        kbf = p1w.tile([128, SQT], BF16, tag="kbf")
                        nc.vector.tensor_scalar(
                            kbf, ps_k, 1.0 / WSCALE,
                            bqk_sb[:, HPC + h : HPC + h + 1],
                            mybir.AluOpType.mult, mybir.AluOpType.add,
                        )
                        nc.gpsimd.tensor_copy(khl_sb[:, 0, h, ssl], kbf)
                        nc.gpsimd.tensor_tensor(
                            out=khl_sb[:, 1, h, ssl], in0=kbf,
                            in1=khl_sb[:, 0, h, ssl],
                            op=mybir.AluOpType.subtract,
                        )

                    def k_mm(ps_k, h, ssl, dtp, start, stop):
                        nc.tensor.matmul(
                            ps_k,
                            wk_sb[:, dtp : dtp + 2, h * HD : (h + 1) * HD],
                            xh_sb[:, dtp : dtp + 2, ssl],
                            start=start, stop=stop, perf_mode=DR,
                        )

                    # first wave of 8 K tiles emitted chunk-major ACROSS
                    # tiles (all 8 psum banks open) so each arriving x chunk
                    # finds ready PE work; later tiles run dense.
                    wave = []
                    for q in range(2):
                        ssl = slice(q * SQT, q * SQT + SQT)
                        for h in range(HPC):
                            ps_k = qkvps.tile([128, SQT], FD32, tag="qkvps")
                            wave.append((ps_k, h, ssl))
                    for c4 in range(4):
                        for ps_k, h, ssl in wave:
                            for dtp in (4 * c4, 4 * c4 + 2):
                                k_mm(ps_k, h, ssl, dtp,
                                     start=(dtp == 0), stop=(dtp == NDT - 2))
                    for ps_k, h, ssl in wave:
                        k_evac(ps_k, h, ssl)
                    for q in range(2, 4):
                        ssl = slice(q * SQT, q * SQT + SQT)
                        for h in range(HPC):
                            ps_k = qkvps.tile([128, SQT], FD32, tag="qkvps")
                            for dtp in range(0, NDT, 2):
                                k_mm(ps_k, h, ssl, dtp,
                                     start=(dtp == 0), stop=(dtp == NDT - 2))
                            k_evac(ps_k, h, ssl)
                    for q in range(4):  # V tiles
                        sq0 = q * SQT
                        ssl = slice(sq0, sq0 + SQT)
                        for sc in range(4):
                            ps_v = qkvps.tile([128, SQT], FD32, tag="qkvps")
                            csl = slice(sq0 + sc * 128, sq0 + (sc + 1) * 128)
                            for dt in range(NDT):
                                nc.tensor.matmul(
                                    ps_v,
                                    xh_sb[:, dt, csl]
                                    .unsqueeze(1).broadcast_to([128, 2, 128]),
                                    wv_sb[:, dt, :, :],
                                    start=(dt == 0), stop=(dt == NDT - 1),
                                    perf_mode=DR,
                                )
                            nc.vector.tensor_scalar_mul(
                                v_sb[:, q * 4 + sc, :], ps_v, 1.0 / WSCALE
                            )
                    sq0 = QJ_ORDER[0] * SQT
                    ssl = slice(sq0, sq0 + SQT)
                    for h in range(HPC):  # Q for the first attention block
                        ps_q = qkvps.tile([128, SQT], FD32, tag="qkvps")
                        dr_proj(ps_q, wq_sb, [xh_sb[:, :, ssl]],
                                slice(h * HD, (h + 1) * HD))
                        nc.vector.tensor_scalar(
                            qt_sb[:, h, ssl], ps_q, 1.0 / WSCALE,
                            bqk_sb[:, h : h + 1],
                            mybir.AluOpType.mult, mybir.AluOpType.add,
                        )

                # ---- phases 2+3: Q projection + attention + dense, per sq
                with (
                    tc.tile_pool(name="work", bufs=WORK_BUFS) as work,
                    tc.tile_pool(name="ctxtp", bufs=CTXT_BUFS) as ctxtp,
                    tc.tile_pool(name="outsb", bufs=OUTSB_BUFS) as outsb,
                    tc.tile_pool(name="maskp", bufs=2) as maskp,
                ):

                    def emit_dense(sq0, ctxt_sb, pool, tag="dps", split=False):
                        for sc in range(4):
                            out_sb = outsb.tile([128, D], BF16, name="out_sb")
                            for do in range(4):
                                o_ps = pool.tile(
                                    [128, 512], FD32, tag=tag, name="o_ps"
                                )
                                dsl = slice(do * 512, (do + 1) * 512)
                                csl = slice(sc * 128, (sc + 1) * 128)
                                # ctx8 (wh+wl): the W_dense quantization is
                                # cancelled by the hi/lo split; ctx8 noise is
                                # within the measured error budget
                                terms = [(0, wdh_sb), (0, wdl_sb)]
                                n = 0
                                for cp, wd_sb in terms:
                                    for hp in (0, 2):
                                        nc.tensor.matmul(
                                            o_ps,
                                            ctxt_sb[:, cp, hp : hp + 2, csl],
                                            wd_sb[:, hp : hp + 2, dsl],
                                            start=(n == 0), stop=(n == 3),
                                            perf_mode=DR,
                                        )
                                        n += 1
                                if do % 2 == 0:
                                    nc.vector.tensor_scalar_mul(
                                        out_sb[:, dsl], o_ps, 1.0 / WSCALE
                                    )
                                else:
                                    nc.scalar.mul(out_sb[:, dsl], o_ps, 1.0 / WSCALE)
                                if split and do % 2 == 1:
                                    r0 = sq0 + sc * 128
                                    nc.sync.dma_start(
                                        out=outp[r0 : r0 + 128,
                                                 (do - 1) * 512 : (do + 1) * 512],
                                        in_=out_sb[:, (do - 1) * 512 : (do + 1) * 512],
                                    )
                            if not split:
                                r0 = sq0 + sc * 128
                                nc.sync.dma_start(
                                    out=outp[r0 : r0 + 128, :], in_=out_sb
                                )

                    last_ctxt = None
                    with (
                        tc.tile_pool(name="scps", bufs=PSUM_SCPS, space="PSUM") as scps,
                        tc.tile_pool(name="ctxps", bufs=PSUM_CTXPS, space="PSUM") as ctxps,
                        tc.tile_pool(name="smps", bufs=PSUM_SMPS, space="PSUM") as smps,
                        tc.tile_pool(name="dps", bufs=PSUM_DPS, space="PSUM") as dps,
                    ):
                        # deferred-emission queue: the normalization tail of
                        # head h (bc matmul + recip + fp8 split) and the dense
                        # block of each qj are emitted a couple of tiles into
                        # the NEXT head's stream, so the in-order PE never
                        # stalls on the act/DVE evac chains.
                        pending = []

                        def flush_pending():
                            for f in pending:
                                f()
                            pending.clear()

                        def make_norm(ctx_ps, sm_sb, ctxt_sb, h):
                            def norm():
                                bc_ps = scps.tile([128, SQT], FD32, tag="scps")
                                nc.tensor.matmul(bc_ps, ones1b_sb, sm_sb,
                                                 start=True, stop=True)
                                rc_sb = work.tile([128, SQT], FD32, tag="rc")
                                nc.vector.reciprocal(rc_sb, bc_ps)
                                ctxnb = work.tile([128, SQT], BF16, tag="ctxnb")
                                nc.vector.tensor_tensor(
                                    out=ctxnb, in0=ctx_ps, in1=rc_sb,
                                    op=mybir.AluOpType.mult,
                                )
                                nc.gpsimd.tensor_copy(ctxt_sb[:, 0, h, :], ctxnb)
                            return norm

                        for qj in QJ_ORDER:
                            sq0 = qj * SQT
                            ssl = slice(sq0, sq0 + SQT)
                            if qj != QJ_ORDER[0]:
                                for h in range(HPC):
                                    ps_q = scps.tile([128, SQT], FD32, tag="scps",
                                                    name="ps_q")
                                    dr_proj(ps_q, wq_sb, [xh_sb[:, :, ssl]],
                                            slice(h * HD, (h + 1) * HD))
                                    nc.vector.tensor_scalar(
                                        qt_sb[:, h, ssl], ps_q, 1.0 / WSCALE,
                                        bqk_sb[:, h : h + 1],
                                        mybir.AluOpType.mult, mybir.AluOpType.add,
                                    )
                            ctxt_sb = ctxtp.tile([128, 2, HPC, SQT], FP8)
                            for h in range(HPC):
                                plan = plans[h]
                                ki_list = [
                                    ki for ki in range(NKT)
                                    if plan[qj][ki] is not None
                                ]
                                ctx_ps = ctxps.tile([128, SQT], FD32, tag="ctxps")
                                sm_ps = smps.tile([1, SQT], FD32, tag="smps")
                                for n, ki in enumerate(ki_list):
                                    kind, off, end = plan[qj][ki]
                                    w = end - off
                                    q0o = sq0 + off
                                    s_ps = scps.tile([128, SQT], FD32, tag="scps")
                                    if not fold[h]:
                                        # rank-1 shift: 128 * (-c/128)
                                        nc.tensor.matmul(
                                            s_ps[:, off:end],
                                            shiftw_sb,
                                            negc_sb[0:1,
                                                    h * S + q0o : h * S + sq0 + end]
                                            .unsqueeze(1).broadcast_to([1, 2, w]),
                                            start=True,
                                            stop=False,
                                            perf_mode=DR,
                                        )
                                    # scores: (K_hi+K_lo) @ Q^T, one DR pass;
                                    # both rhs planes read the SAME Q block
                                    nc.tensor.matmul(
                                        s_ps[:, off:end],
                                        khl_sb[:, :, h, ki * 128 : (ki + 1) * 128],
                                        qt_sb[:, h, q0o : sq0 + end]
                                        .unsqueeze(1).broadcast_to([128, 2, w]),
                                        start=fold[h],
                                        stop=True,
                                        perf_mode=DR,
                                    )
                                    if kind == "data":
                                        mk_sb = maskp.tile([128, SQT], FD32, tag="mask")
                                        nc.sync.dma_start(
                                            out=mk_sb,
                                            in_=maskt[
                                                ki * 128 : (ki + 1) * 128, ssl
                                            ],
                                        )
                                        nc.vector.tensor_tensor(
                                            out=s_ps, in0=s_ps, in1=mk_sb,
                                            op=mybir.AluOpType.add,
                                        )
                                    pt_sb = work.tile([128, SQT], BF16, tag="pt",
                                                      bufs=PT_BUFS)
                                    nc.scalar.activation(
                                        pt_sb[:, 0:w],
                                        s_ps[:, off:end],
                                        mybir.ActivationFunctionType.Exp,
                                        bias=alib_sb[:, h * NKT + ki : h * NKT + ki + 1],
                                        scale=INV_NORM,
                                    )
                                    if kind == "pat":
                                        # zero the sk>sq region post-exp
                                        nc.gpsimd.affine_select(
                                            out=pt_sb[:, 0:w],
                                            in_=pt_sb[:, 0:w],
                                            compare_op=mybir.AluOpType.is_ge,
                                            fill=0.0,
                                            base=0,
                                            pattern=[[1, w]],
                                            channel_multiplier=-1,
                                        )
                                    nc.tensor.matmul(
                                        ctx_ps[:, off:end],
                                        v_sb[:, ki, h * HD : (h + 1) * HD],
                                        pt_sb[:, 0:w],
                                        start=(n == 0),
                                        stop=(n == len(ki_list) - 1),
                                    )
                                    nc.tensor.matmul(
                                        sm_ps[0:1, off:end],
                                        onesp1_sb,
                                        pt_sb[:, 0:w],
                                        start=(n == 0),
                                        stop=(n == len(ki_list) - 1),
                                    )
                                    if n == FLUSH_N:
                                        flush_pending()
                                # sums to sbuf now (act-side, no PE stall);
                                # the rest of the chain is deferred
                                sm_sb = work.tile([1, SQT], BF16, tag="sm")
                                (nc.scalar.copy if SM_ON_ACT
                                 else nc.vector.tensor_copy)(sm_sb, sm_ps)
                                pending.append(
                                    make_norm(ctx_ps, sm_sb, ctxt_sb, h)
                                )
                            if qj != QJ_ORDER[-1]:
                                pending.append(
                                    (lambda s, c: lambda: emit_dense(s, c, dps))(
                                        sq0, ctxt_sb
                                    )
                                )
                            else:
                                last_ctxt = ctxt_sb
                        flush_pending()  # remaining norms (need scps)

                    # tail: dense for the last block with full psum freedom
                    with tc.tile_pool(name="dps2", bufs=4, space="PSUM") as dps2:
                        emit_dense(QJ_ORDER[-1] * SQT, last_ctxt, dps2,
                                   split=True)

    _split_multi_waits(nc)
    return nc


_PROGRAM_CACHE = {}


def _get_program(mode, trim=True):
    key = (mode, trim)
    if key not in _PROGRAM_CACHE:
        _PROGRAM_CACHE[key] = _build_program(mode, trim)
    return _PROGRAM_CACHE[key]


def _bloom_alibi_ok(alibi):
    """True iff alibi matches the standard BLOOM pattern the tile trims
    assume (slopes base**(h+1) times position)."""
    slopes = SLOPE_BASE ** np.arange(1, H + 1, dtype=np.float32)
    pos = np.arange(S, dtype=np.float32)
    exp = np.broadcast_to(
        (slopes[:, None] * pos[None, :])[None], (B, H, S)
    ).reshape(B * H, 1, S)
    return np.allclose(alibi, exp, rtol=1e-4, atol=1e-3)


def _classify_mask(mask):
    """mask: [B, 1, S, S] float32 -> 'none' | 'causal' | 'data'."""
    if not np.any(mask):
        return "none"
    tril = np.tril(np.ones((S, S), dtype=bool))
    for b in range(mask.shape[0]):
        m = mask[b, 0]
        if not (np.all(m[tril] == 0.0) and np.all(m[~tril] <= -1.0e8)):
            return "data"
    return "causal"


def kernel(
    hidden_states,
    residual,
    alibi,
    attention_mask,
    W_qkv,
    b_qkv,
    W_dense,
    b_dense,
):
    hidden_states = np.asarray(hidden_states, dtype=np.float32)
    residual = np.asarray(residual, dtype=np.float32)
    alibi = np.asarray(alibi, dtype=np.float32)
    attention_mask = np.asarray(attention_mask, dtype=np.float32)
    W_qkv = np.asarray(W_qkv, dtype=np.float32)
    b_qkv = np.asarray(b_qkv, dtype=np.float32)
    W_dense = np.asarray(W_dense, dtype=np.float32)
    b_dense = np.asarray(b_dense, dtype=np.float32)

    mode = _classify_mask(attention_mask)
    trim = bool(_bloom_alibi_ok(alibi))
    nc = _get_program(mode, trim)

    # W_qkv row blocks per head: rows h*384+[0:128) = q, +128 k, +256 v
    wq = W_qkv.reshape(H, 3, HD, D)[:, 0]  # [H, HD, D]
    wk = W_qkv.reshape(H, 3, HD, D)[:, 1]
    wv = W_qkv.reshape(H, 3, HD, D)[:, 2]
    bq = b_qkv.reshape(H, 3, HD)[:, 0]  # [H, HD]
    bk = b_qkv.reshape(H, 3, HD)[:, 1]
    bv = b_qkv.reshape(H, 3, HD)[:, 2]

    onesp1 = np.ones((128, 1), dtype=BF16_NP)
    ones1b = np.ones((1, 128), dtype=BF16_NP)
    shiftw_np = np.concatenate(
        [np.full((1, 128), SHIFT_ONES, np.float32), np.zeros((1, 128), np.float32)],
        axis=1,
    ).astype(F8_NP)

    xh_by_batch = []
    for b in range(B):
        xt = np.ascontiguousarray(hidden_states[b].T)
        xh_by_batch.append(xt.astype(F8_NP))

    def _hl_split(w):  # [D, F] fp32*WSCALE -> [D, 2F] fp8 hi|lo interleaved
        hi = w.astype(F8_NP)
        lo = (w - hi.astype(np.float32)).astype(F8_NP)
        return np.ascontiguousarray(
            np.stack([hi, lo], axis=1).reshape(w.shape[0], 2 * w.shape[1])
        )

    maskt_by_batch = None
    if mode == "data":
        # Clamp very-negative mask values: anything <= -190 already gives an
        # exact 0 after exp, and bounding |c| keeps the shift vector sane.
        # Clamp must exceed the full ALiBi span (~1450): a masked far key
        # has a[k] up to that much ABOVE the allowed maximum, and the clamp
        # has to keep a[k] + mask at least ~100 below c for an exact 0.
        attention_mask = np.maximum(attention_mask, np.float32(-1800.0))
        # the exp act multiplies the whole psum by INV_NORM, so pre-divide
        maskt_by_batch = [
            np.ascontiguousarray(attention_mask[b, 0].T / INV_NORM).astype(np.float32)
            for b in range(B)
        ]

    in_maps = []
    for c in range(NCORES):
        b = c // 4
        g = c % 4
        heads = [4 * j + g for j in range(HPC)]  # slot j -> head 4j+g

        wq_c = wq[heads].reshape(HPC * HD, D)  # [512, D]
        wk_c = wk[heads].reshape(HPC * HD, D)
        wv_c = wv[heads].reshape(HPC * HD, D)
        wd_c = W_dense[:, [h * HD + i for h in heads for i in range(HD)]]  # [D, 512]

        bqk_np = np.stack(
            [bq[h] for h in heads] + [bk[h] for h in heads], axis=1
        ).astype(np.float32)  # [128, 8]

        # per-head alibi columns [128, HPC*NKT] and shift c
        al = np.empty((128, HPC * NKT), dtype=np.float32)
        negc_np = np.empty((HPC, S), dtype=np.float32)
        fold = _shift_fold(mode, trim)
        for hl, h in enumerate(heads):
            a = alibi[b * H + h, 0]  # [S]
            if mode == "none":
                c_vec = np.full(S, a.max(), dtype=np.float32)
            elif mode == "causal":
                c_vec = np.maximum.accumulate(a)
            else:
                c_vec = (a[None, :] + attention_mask[b, 0]).max(axis=1)
            bias_cols = a.reshape(NKT, 128).T
            if fold[hl]:
                # constant shift folded into the fp32 act bias
                bias_cols = bias_cols - (c_vec.max() + c_vec.min()) / 2.0
                negc_np[hl] = 0.0
            else:
                negc_np[hl] = -c_vec / (INV_NORM * SHIFT_ONES)
            al[:, hl * NKT : (hl + 1) * NKT] = bias_cols

        wdt_c = np.ascontiguousarray(wd_c.T) * WSCALE  # [512, D]
        wdh_np = wdt_c.astype(F8_NP)
        wdl_np = (wdt_c - wdh_np.astype(np.float32)).astype(F8_NP)

        im = {
            "xh": xh_by_batch[b],
            "wqt": np.ascontiguousarray(wq_c.T * WSCALE).astype(F8_NP),
            "wkt": np.ascontiguousarray(wk_c.T * WSCALE).astype(F8_NP),
            "wvt": _hl_split(np.ascontiguousarray(wv_c.T) * WSCALE),
            "wdh": wdh_np,
            "wdl": wdl_np,
            "bqk": bqk_np,
            "alib": al,
            "negc": negc_np.reshape(1, HPC * S).astype(F8_NP),
            "shiftw": shiftw_np,
            "onesp1": onesp1,
            "ones1b": ones1b,
        }
        if mode == "data":
            im["maskt"] = maskt_by_batch[b]
        in_maps.append(im)

    res = None
    last_exc = None
    for attempt in range(3):
        try:
            res = bass_utils.run_bass_kernel_spmd(
                nc, in_maps, core_ids=list(range(NCORES))
            )
            break
        except Exception as e:  # transient device wedges (NRT_EXEC_*) happen
            last_exc = e
            time.sleep(2.0 * (attempt + 1))
    if res is None:
        raise last_exc

    # host-side v-bias fold: (ctx + bv) @ Wd^T = ctx @ Wd^T + bv @ Wd^T
    bv_flat = bv.reshape(D)
    bv_term = bv_flat @ W_dense.T  # [D]

    out = np.empty((B, S, D), dtype=np.float32)
    for b in range(B):
        acc = b_dense[None, :] + bv_term[None, :] + residual[b]
        for g in range(4):
            acc = acc + res.results[b * 4 + g]["outp"].astype(np.float32)
        out[b] = acc
    return out


# revision 50
# speedup vs baseline: 1.0618x; 1.0618x over previous
"""BLOOM attention block (B=2, S=2048, D=2048, H=16) on 8 Trainium2 NeuronCores.

Sharding: core c handles batch b=c//4 and head group g=c%4; head slot j on
core (b,g) computes global head 4*j+g, so every slot sees a narrow ALiBi
slope band (slot 0 = steepest slopes on every core) and the per-slot tile
plan can exploit ALiBi decay uniformly across the SPMD cores.

Numerics (fp8 = e4m3, scale 256 on weights):
  - Q = x8 @ wq8 (fp8 DoubleRow), stored fp8.
  - K = (x_hi+x_lo) @ wk8 (fp8 DR, x error cancelled), stored as an fp8
    hi+lo pair; the scores matmul contracts both planes in one DoubleRow
    pass, so K enters at ~bf16 fidelity at half the bf16 cost.
  - V = (x_hi+x_lo) @ wv8, stored bf16.
  - scores^T[sk,sq] = (K_hi+K_lo) @ Q^T via one DR matmul; the analytic
    per-column shift -c[sq] (host cummax of alibi[+mask]) is injected by a
    rank-1 fp8 DR matmul (128 * c/128); its quantization error is constant
    per column and cancels exactly in the softmax normalization.
  - exp on ACT with bias=alibi col (fp32) and scale=INV_NORM; pt in bf16.
  - ctx^T = V^T @ P^T and column sums (ones matmul) in bf16; the qkv v-bias
    is folded host-side (bv @ W_dense^T added to the output) since
    sum(P)=1 after normalization.
  - dense: ctx stored fp8; W_dense^T stored as fp8 hi+lo pair; 4 DR
    matmuls per psum tile (heads paired for the hi plane and for the lo
    plane), output bf16.
  - The far-distance clean tiles of slot 0 (steep slopes) are skipped
    entirely: exp(score + a[k]-a[sq]) underflows bf16 to exactly 0 there.
"""

import math
import time

import numpy as np

import bass_rust
import concourse.bass as bass
import concourse.mybir as mybir
import concourse.tile as tile
from concourse import bass_utils

import ml_dtypes

BF16_NP = ml_dtypes.bfloat16
F8_NP = ml_dtypes.float8_e4m3

B, S, D, H = 2, 2048, 2048, 16
HD = D // H  # 128
INV_NORM = 1.0 / math.sqrt(HD)
NCORES = 8
HPC = 4  # head slots per core
SQT = 512  # sq tile width (free dim of transposed score tiles)
NQT = S // SQT  # 4
NKT = S // 128  # 16 sk tiles
NDT = D // 128  # 16 contraction tiles
FD32 = mybir.dt.float32
BF16 = mybir.dt.bfloat16
FP8 = mybir.dt.float8e4
DR = mybir.MatmulPerfMode.DoubleRow
NEG_BIG = -1.0e9
WSCALE = 256.0
SHIFT_ONES = 128.0
QJ_ORDER = [3, 2, 1, 0]
PSUM_QKV = 8
PSUM_SCPS = 3
PSUM_CTXPS = 2
PSUM_SMPS = 2
PSUM_DPS = 1
PT_BUFS = 4
WORK_BUFS = 4
QX2_BUFS = 2
FLUSH_N = 2
SM_ON_ACT = False
CTXT_BUFS = 2
OUTSB_BUFS = 4
# ALiBi-decay cutoff distance per head slot (slot j holds heads 4j..4j+3;
# the shallowest slope in slot j is head 4j+3). A dropped key at distance
# > dist has weight < e^{12-52} = e^-40 of the column's max term (scores
# span +-6), i.e. zero at fp32 accumulation precision.
SLOPE_BASE = 2.0 ** (-(2.0 ** -(math.log2(H) - 3)))
SLOT_SKIP_DIST = [52.0 / (SLOPE_BASE ** (4 * j + 4)) for j in range(HPC)]


def _split_multi_waits(nc):
    """This toolchain's walrus accepts at most ONE sync wait per instruction;
    Tile emits multi-wait instructions. Move extra waits onto preceding NOPs
    on the same engine (waits execute in stream order, so semantics hold)."""
    for fn in nc.m.functions:
        for bb in fn.blocks:
            insts = bb.instructions
            i = 0
            while i < len(insts):
                inst = insts[i]
                si = inst.sync_info
                if si is not None and len(si.on_wait) > 1:
                    waits = list(si.on_wait)
                    carriers = []
                    for k, w in enumerate(waits[:-1]):
                        nop = mybir.InstNoOp(name=f"{inst.name}_sw{k}", ins=[], outs=[])
                        nop.engine = inst.engine
                        nop.sync_info = bass_rust.SyncInfo(on_wait=[w], on_update=[])
                        nc.register_instruction(nop, overwrite=True)
                        carriers.append(nop)
                    inst.sync_info = bass_rust.SyncInfo(
                        on_wait=[waits[-1]], on_update=si.on_update
                    )
                    insts[i:i] = carriers
                    i += len(carriers)
                i += 1


def _tile_plan(mode, trim):
    """plan[slot][qj][ki] is None (skip) or (kind, off, end): valid sq
    columns [off, end) of the tile. 'pat' needs the post-exp triangle zero.
    In causal mode, columns beyond the slot's ALiBi cutoff distance are
    trimmed away (right cut) and fully-cut tiles are skipped."""
    plans = []
    for slot in range(HPC):
        plan = []
        for qj in range(NQT):
            row = []
            for ki in range(NKT):
                if mode == "none":
                    row.append(("clean", 0, SQT))
                elif mode == "data":
                    row.append(("data", 0, SQT))
                else:  # causal: keys sk <= queries sq
                    sk_lo, sk_hi = 128 * ki, 128 * ki + 127
                    sq0 = SQT * qj
                    if sk_lo > sq0 + SQT - 1:
                        row.append(None)
                        continue
                    off = max(0, sk_lo - sq0)
                    dist = SLOT_SKIP_DIST[slot] if trim else 10 * S
                    end = min(SQT, sk_hi + int(dist) - sq0 + 1)
                    if end <= off:
                        row.append(None)
                    elif sk_hi <= sq0:
                        row.append(("clean", 0, end))
                    else:
                        row.append(("pat", off, end))
            plan.append(row)
        plans.append(plan)
    return plans


def _shift_fold(mode, trim):
    """Slots whose shift fits entirely in the fp32 act bias: when the
    per-head cummax range is small (shallow ALiBi slopes), subtracting a
    per-head CONSTANT keeps exp args bounded, so the rank-1 shift matmul
    can be dropped (the residual per-column factor cancels in the
    normalization)."""
    if mode == "causal":
        # requires the known BLOOM slope band per slot
        return [False, False, trim, trim]
    if mode == "none":
        return [True] * HPC  # c is a per-head constant for any alibi
    return [False] * HPC


def _build_program(mode, trim):
    """mode in {'none', 'causal', 'data'}; returns the Bass module. trim
    enables the ALiBi-decay tile skips/cuts and per-slot shift folding,
    valid only for the standard BLOOM alibi."""
    plans = _tile_plan(mode, trim)
    fold = _shift_fold(mode, trim)

    nc = bass.Bass()
    xh = nc.dram_tensor("xh", [D, S], FP8, kind="ExternalInput")
    wqt = nc.dram_tensor("wqt", [D, HPC * HD], FP8, kind="ExternalInput")
    wkt = nc.dram_tensor("wkt", [D, HPC * HD], FP8, kind="ExternalInput")
    wvt = nc.dram_tensor("wvt", [D, 2 * HPC * HD], FP8, kind="ExternalInput")
    wdh = nc.dram_tensor("wdh", [HPC * HD, D], FP8, kind="ExternalInput")
    wdl = nc.dram_tensor("wdl", [HPC * HD, D], FP8, kind="ExternalInput")
    bqk = nc.dram_tensor("bqk", [128, 2 * HPC], FD32, kind="ExternalInput")
    alib = nc.dram_tensor("alib", [128, HPC * NKT], FD32, kind="ExternalInput")
    negc = nc.dram_tensor("negc", [1, HPC * S], FP8, kind="ExternalInput")
    shiftw = nc.dram_tensor("shiftw", [1, 2 * 128], FP8, kind="ExternalInput")
    onesp1 = nc.dram_tensor("onesp1", [128, 1], BF16, kind="ExternalInput")
    ones1b = nc.dram_tensor("ones1b", [1, 128], BF16, kind="ExternalInput")
    maskt = None
    if mode == "data":
        maskt = nc.dram_tensor("maskt", [S, S], FD32, kind="ExternalInput")
    outp = nc.dram_tensor("outp", [S, D], BF16, kind="ExternalOutput")

    with tile.TileContext(nc) as tc:
        with tc.tile_pool(name="persist", bufs=1) as persist:
            # ---- persistent SBUF tensors -------------------------------
            qt_sb = persist.tile([128, HPC + 1, S], FP8)  # +1 junk plane
            khl_sb = persist.tile([128, 2, HPC, S], FP8)  # K hi/lo planes
            v_sb = persist.tile([128, NKT, HPC * HD], BF16)  # V native
            wdh_sb = persist.tile([128, HPC, D], FP8)
            wdl_sb = persist.tile([128, HPC, D], FP8)
            xh_sb = persist.tile([128, NDT, S], FP8)  # resident x_hi
            bqk_sb = persist.tile([128, 2 * HPC], FD32)
            alib_sb = persist.tile([128, HPC * NKT], FD32)
            negc_sb = persist.tile([1, HPC * S], FP8)
            shiftw_sb = persist.tile([1, 2, 128], FP8)
            onesp1_sb = persist.tile([128, 1], BF16)
            ones1b_sb = persist.tile([1, 128], BF16)

            def load_consts():
                # issued AFTER the bulk x/w loads: tiny DMAs must not
                # head-of-line-block the HWDGE dispatch queue at startup
                nc.gpsimd.dma_start(out=bqk_sb, in_=bqk[:])
                nc.gpsimd.dma_start(out=alib_sb, in_=alib[:])
                nc.gpsimd.dma_start(out=negc_sb, in_=negc[:])
                nc.gpsimd.dma_start(
                    out=shiftw_sb, in_=shiftw.rearrange("p (k j) -> p k j", k=2)
                )
                nc.gpsimd.dma_start(out=onesp1_sb, in_=onesp1[:])
                nc.gpsimd.dma_start(out=ones1b_sb, in_=ones1b[:])
                # junk plane read by the slot-3 scores matmul must be finite
                # (0 * NaN = NaN on the PE); placed after the bulk loads so
                # it never delays the startup-critical DMAs
                nc.gpsimd.memset(qt_sb[:, HPC, :], 0.0)


            xh_r = xh.rearrange("(dt p) s -> p dt s", p=128)
            wqt_r = wqt.rearrange("(dt p) f -> p dt f", p=128)
            wkt_r = wkt.rearrange("(dt p) f -> p dt f", p=128)
            wvt_r = wvt.rearrange("(dt p) (k f) -> p dt k f", p=128, k=2)

            def dr_proj(ps, w_sb, x_tiles, hs, first_start=True):
                """psum += sum over x planes/dt pairs of w^T x (DoubleRow).
                Emission is chunk-major (dt groups of 4) so the matmuls
                consume the x DMA chunks in arrival order."""
                order = []
                for c4 in range(4):
                    for xp in x_tiles:
                        for dtp in (4 * c4, 4 * c4 + 2):
                            order.append((xp, dtp))
                for n, (xp, dtp) in enumerate(order):
                    nc.tensor.matmul(
                        ps,
                        w_sb[:, dtp : dtp + 2, hs],
                        xp[:, dtp : dtp + 2, :],
                        start=(n == 0 and first_start),
                        stop=(n == len(order) - 1),
                        perf_mode=DR,
                    )

            # ---- phase 1: K+V projection (weights split hi/lo fp8, so
            # each dt needs ONE DoubleRow pass; x_hi is the only x input)
            with tc.tile_pool(name="wqp", bufs=1) as wqp:
                wq_sb = wqp.tile([128, NDT, HPC * HD], FP8)
                with (
                    tc.tile_pool(name="qkvw", bufs=1) as qkvw,
                    tc.tile_pool(name="p1w", bufs=4) as p1w,
                    tc.tile_pool(name="qkvps", bufs=PSUM_QKV, space="PSUM") as qkvps,
                ):
                    wk_sb = qkvw.tile([128, NDT, HPC * HD], FP8)
                    wv_sb = qkvw.tile([128, NDT, 2, HPC * HD], FP8)
                    # fine-grained chunked loads; V weights ride with the x
                    # chunks (V tiles have 2x the PE work density per chunk)
                    for dsl in (slice(0, 2), slice(2, 4), slice(4, 8),
                                slice(8, 12), slice(12, 16)):
                        nc.sync.dma_start(
                            out=wv_sb[:, dsl, :, :], in_=wvt_r[:, dsl, :, :]
                        )
                        nc.gpsimd.dma_start(out=xh_sb[:, dsl, :], in_=xh_r[:, dsl, :])
                    load_consts()
                    for c2 in range(2):
                        dsl = slice(c2 * 8, (c2 + 1) * 8)
                        nc.sync.dma_start(out=wk_sb[:, dsl, :], in_=wkt_r[:, dsl, :])
                    for c2 in range(2):
                        dsl = slice(c2 * 8, (c2 + 1) * 8)
                        nc.sync.dma_start(out=wq_sb[:, dsl, :], in_=wqt_r[:, dsl, :])
                    for c2 in range(2):
                        nc.sync.dma_start(
                            out=wdh_sb[:, c2 * 2 : c2 * 2 + 2, :],
                            in_=wdh.rearrange("(h p) o -> p h o", p=128)[
                                :, c2 * 2 : c2 * 2 + 2, :
                            ],
                        )
                        nc.sync.dma_start(
                            out=wdl_sb[:, c2 * 2 : c2 * 2 + 2, :],
                            in_=wdl.rearrange("(h p) o -> p h o", p=128)[
                                :, c2 * 2 : c2 * 2 + 2, :
                            ],
                        )

                    def k_evac(ps_k, h, ssl):
                        # one op per engine so no single engine paces the
                        # K stretch (phase-1 Act is otherwise idle)
                        kbf = p1w.tile([128, SQT], BF16, tag="kbf")
                        nc.vector.tensor_scalar(
                            kbf, ps_k, 1.0 / WSCALE,
                            bqk_sb[:, HPC + h : HPC + h + 1],
                            mybir.AluOpType.mult, mybir.AluOpType.add,
                        )
                        nc.scalar.copy(khl_sb[:, 0, h, ssl], kbf)
                        nc.gpsimd.tensor_tensor(
                            out=khl_sb[:, 1, h, ssl], in0=kbf,
                            in1=khl_sb[:, 0, h, ssl],
                            op=mybir.AluOpType.subtract,
                        )

                    def k_mm(ps_k, h, ssl, dtp, start, stop):
                        nc.tensor.matmul(
                            ps_k,
                            wk_sb[:, dtp : dtp + 2, h * HD : (h + 1) * HD],
                            xh_sb[:, dtp : dtp + 2, ssl],
                            start=start, stop=stop, perf_mode=DR,
                        )

                    # wave of 8 V tiles emitted chunk-major ACROSS tiles
                    # (8 psum banks open): V has 2 DR per dt (hi/lo weight
                    # split), the highest ready-work density per x chunk.
                    def v_mm(ps_v, csl, dt, start, stop):
                        nc.tensor.matmul(
                            ps_v,
                            xh_sb[:, dt, csl]
                            .unsqueeze(1).broadcast_to([128, 2, 128]),
                            wv_sb[:, dt, :, :],
                            start=start, stop=stop, perf_mode=DR,
                        )

                    vwave = []
                    for t in range(8):
                        ps_v = qkvps.tile([128, SQT], FD32, tag="qkvps")
                        vwave.append((ps_v, t, slice(t * 128, (t + 1) * 128)))
                    for c4 in range(4):
                        for ps_v, t, csl in vwave:
                            for dt in range(4 * c4, 4 * c4 + 4):
                                v_mm(ps_v, csl, dt,
                                     start=(dt == 0), stop=(dt == NDT - 1))
                    for ps_v, t, csl in vwave:
                        nc.vector.tensor_scalar_mul(
                            v_sb[:, t, :], ps_v, 1.0 / WSCALE
                        )
                    for t in range(8, 16):
                        ps_v = qkvps.tile([128, SQT], FD32, tag="qkvps")
                        csl = slice(t * 128, (t + 1) * 128)
                        for dt in range(NDT):
                            v_mm(ps_v, csl, dt,
                                 start=(dt == 0), stop=(dt == NDT - 1))
                        nc.vector.tensor_scalar_mul(
                            v_sb[:, t, :], ps_v, 1.0 / WSCALE
                        )
                    for q in range(4):  # K tiles (dense; weights resident)
                        ssl = slice(q * SQT, q * SQT + SQT)
                        for h in range(HPC):
                            ps_k = qkvps.tile([128, SQT], FD32, tag="qkvps")
                            for dtp in range(0, NDT, 2):
                                k_mm(ps_k, h, ssl, dtp,
                                     start=(dtp == 0), stop=(dtp == NDT - 2))
                            k_evac(ps_k, h, ssl)
                    sq0 = QJ_ORDER[0] * SQT
                    ssl = slice(sq0, sq0 + SQT)
                    for h in range(HPC):  # Q for the first attention block
                        ps_q = qkvps.tile([128, SQT], FD32, tag="qkvps")
                        dr_proj(ps_q, wq_sb, [xh_sb[:, :, ssl]],
                                slice(h * HD, (h + 1) * HD))
                        # Act-side evac: DVE has a V/K evac backlog here and
                        # the first attention block waits on this
                        nc.scalar.tensor_scalar(
                            qt_sb[:, h, ssl], ps_q, 1.0 / WSCALE,
                            bqk_sb[:, h : h + 1],
                            mybir.AluOpType.mult, mybir.AluOpType.add,
                        )

                # ---- phases 2+3: Q projection + attention + dense, per sq
                with (
                    tc.tile_pool(name="work", bufs=WORK_BUFS) as work,
                    tc.tile_pool(name="ctxtp", bufs=CTXT_BUFS) as ctxtp,
                    tc.tile_pool(name="outsb", bufs=OUTSB_BUFS) as outsb,
                    tc.tile_pool(name="maskp", bufs=2) as maskp,
                ):

                    def emit_dense(sq0, ctxt_sb, pool, tag="dps", split=False):
                        for sc in range(4):
                            out_sb = outsb.tile([128, D], BF16, name="out_sb")
                            for do in range(4):
                                o_ps = pool.tile(
                                    [128, 512], FD32, tag=tag, name="o_ps"
                                )
                                dsl = slice(do * 512, (do + 1) * 512)
                                csl = slice(sc * 128, (sc + 1) * 128)
                                # ctx8 (wh+wl): the W_dense quantization is
                                # cancelled by the hi/lo split; ctx8 noise is
                                # within the measured error budget
                                terms = [(0, wdh_sb), (0, wdl_sb)]
                                n = 0
                                for cp, wd_sb in terms:
                                    for hp in (0, 2):
                                        nc.tensor.matmul(
                                            o_ps,
                                            ctxt_sb[:, cp, hp : hp + 2, csl],
                                            wd_sb[:, hp : hp + 2, dsl],
                                            start=(n == 0), stop=(n == 3),
                                            perf_mode=DR,
                                        )
                                        n += 1
                                if do % 2 == 0:
                                    nc.vector.tensor_scalar_mul(
                                        out_sb[:, dsl], o_ps, 1.0 / WSCALE
                                    )
                                else:
                                    nc.scalar.mul(out_sb[:, dsl], o_ps, 1.0 / WSCALE)
                                if split and do % 2 == 1:
                                    r0 = sq0 + sc * 128
                                    nc.sync.dma_start(
                                        out=outp[r0 : r0 + 128,
                                                 (do - 1) * 512 : (do + 1) * 512],
                                        in_=out_sb[:, (do - 1) * 512 : (do + 1) * 512],
                                    )
                            if not split:
                                r0 = sq0 + sc * 128
                                nc.sync.dma_start(
                                    out=outp[r0 : r0 + 128, :], in_=out_sb
                                )

                    last_ctxt = None
                    with (
                        tc.tile_pool(name="scps", bufs=PSUM_SCPS, space="PSUM") as scps,
                        tc.tile_pool(name="ctxps", bufs=PSUM_CTXPS, space="PSUM") as ctxps,
                        tc.tile_pool(name="smps", bufs=PSUM_SMPS, space="PSUM") as smps,
                        tc.tile_pool(name="dps", bufs=PSUM_DPS, space="PSUM") as dps,
                    ):
                        # deferred-emission queue: the normalization tail of
                        # head h (bc matmul + recip + fp8 split) and the dense
                        # block of each qj are emitted a couple of tiles into
                        # the NEXT head's stream, so the in-order PE never
                        # stalls on the act/DVE evac chains.
                        pending = []

                        def flush_pending():
                            for f in pending:
                                f()
                            pending.clear()

                        def make_norm(ctx_ps, sm_sb, ctxt_sb, h):
                            def norm():
                                bc_ps = scps.tile([128, SQT], FD32, tag="scps")
                                nc.tensor.matmul(bc_ps, ones1b_sb, sm_sb,
                                                 start=True, stop=True)
                                rc_sb = work.tile([128, SQT], FD32, tag="rc")
                                nc.vector.reciprocal(rc_sb, bc_ps)
                                ctxnb = work.tile([128, SQT], BF16, tag="ctxnb")
                                nc.vector.tensor_tensor(
                                    out=ctxnb, in0=ctx_ps, in1=rc_sb,
                                    op=mybir.AluOpType.mult,
                                )
                                nc.gpsimd.tensor_copy(ctxt_sb[:, 0, h, :], ctxnb)
                            return norm

                        for qj in QJ_ORDER:
                            sq0 = qj * SQT
                            ssl = slice(sq0, sq0 + SQT)
                            if qj != QJ_ORDER[0]:
                                for h in range(HPC):
                                    ps_q = scps.tile([128, SQT], FD32, tag="scps",
                                                    name="ps_q")
                                    dr_proj(ps_q, wq_sb, [xh_sb[:, :, ssl]],
                                            slice(h * HD, (h + 1) * HD))
                                    nc.vector.tensor_scalar(
                                        qt_sb[:, h, ssl], ps_q, 1.0 / WSCALE,
                                        bqk_sb[:, h : h + 1],
                                        mybir.AluOpType.mult, mybir.AluOpType.add,
                                    )
                            ctxt_sb = ctxtp.tile([128, 2, HPC, SQT], FP8)
                            for h in range(HPC):
                                plan = plans[h]
                                ki_list = [
                                    ki for ki in range(NKT)
                                    if plan[qj][ki] is not None
                                ]
                                ctx_ps = ctxps.tile([128, SQT], FD32, tag="ctxps")
                                sm_ps = smps.tile([1, SQT], FD32, tag="smps")
                                for n, ki in enumerate(ki_list):
                                    kind, off, end = plan[qj][ki]
                                    w = end - off
                                    q0o = sq0 + off
                                    s_ps = scps.tile([128, SQT], FD32, tag="scps")
                                    if not fold[h]:
                                        # rank-1 shift: 128 * (-c/128)
                                        nc.tensor.matmul(
                                            s_ps[:, off:end],
                                            shiftw_sb,
                                            negc_sb[0:1,
                                                    h * S + q0o : h * S + sq0 + end]
                                            .unsqueeze(1).broadcast_to([1, 2, w]),
                                            start=True,
                                            stop=False,
                                            perf_mode=DR,
                                        )
                                    # scores: (K_hi+K_lo) @ Q^T, one DR pass;
                                    # both rhs planes read the SAME Q block
                                    nc.tensor.matmul(
                                        s_ps[:, off:end],
                                        khl_sb[:, :, h, ki * 128 : (ki + 1) * 128],
                                        qt_sb[:, h, q0o : sq0 + end]
                                        .unsqueeze(1).broadcast_to([128, 2, w]),
                                        start=fold[h],
                                        stop=True,
                                        perf_mode=DR,
                                    )
                                    if kind == "data":
                                        mk_sb = maskp.tile([128, SQT], FD32, tag="mask")
                                        nc.sync.dma_start(
                                            out=mk_sb,
                                            in_=maskt[
                                                ki * 128 : (ki + 1) * 128, ssl
                                            ],
                                        )
                                        nc.vector.tensor_tensor(
                                            out=s_ps, in0=s_ps, in1=mk_sb,
                                            op=mybir.AluOpType.add,
                                        )
                                    pt_sb = work.tile([128, SQT], BF16, tag="pt",
                                                      bufs=PT_BUFS)
                                    nc.scalar.activation(
                                        pt_sb[:, 0:w],
                                        s_ps[:, off:end],
                                        mybir.ActivationFunctionType.Exp,
                                        bias=alib_sb[:, h * NKT + ki : h * NKT + ki + 1],
                                        scale=INV_NORM,
                                    )
                                    if kind == "pat":
                                        # zero the sk>sq region post-exp
                                        nc.gpsimd.affine_select(
                                            out=pt_sb[:, 0:w],
                                            in_=pt_sb[:, 0:w],
                                            compare_op=mybir.AluOpType.is_ge,
                                            fill=0.0,
                                            base=0,
                                            pattern=[[1, w]],
                                            channel_multiplier=-1,
                                        )
                                    nc.tensor.matmul(
                                        ctx_ps[:, off:end],
                                        v_sb[:, ki, h * HD : (h + 1) * HD],
                                        pt_sb[:, 0:w],
                                        start=(n == 0),
                                        stop=(n == len(ki_list) - 1),
                                    )
                                    nc.tensor.matmul(
                                        sm_ps[0:1, off:end],
                                        onesp1_sb,
                                        pt_sb[:, 0:w],
                                        start=(n == 0),
                                        stop=(n == len(ki_list) - 1),
                                    )
                                    if n == FLUSH_N:
                                        flush_pending()
                                # sums to sbuf now (act-side, no PE stall);
                                # the rest of the chain is deferred
                                sm_sb = work.tile([1, SQT], BF16, tag="sm")
                                (nc.scalar.copy if SM_ON_ACT
                                 else nc.vector.tensor_copy)(sm_sb, sm_ps)
                                pending.append(
                                    make_norm(ctx_ps, sm_sb, ctxt_sb, h)
                                )
                            if qj != QJ_ORDER[-1]:
                                pending.append(
                                    (lambda s, c: lambda: emit_dense(s, c, dps))(
                                        sq0, ctxt_sb
                                    )
                                )
                            else:
                                last_ctxt = ctxt_sb
                        flush_pending()  # remaining norms (need scps)

                    # tail: dense for the last block with full psum freedom
                    with tc.tile_pool(name="dps2", bufs=4, space="PSUM") as dps2:
                        emit_dense(QJ_ORDER[-1] * SQT, last_ctxt, dps2,
                                   split=True)

    _split_multi_waits(nc)
    return nc


_PROGRAM_CACHE = {}


def _get_program(mode, trim=True):
    key = (mode, trim)
    if key not in _PROGRAM_CACHE:
        _PROGRAM_CACHE[key] = _build_program(mode, trim)
    return _PROGRAM_CACHE[key]


def _bloom_alibi_ok(alibi):
    """True iff alibi matches the standard BLOOM pattern the tile trims
    assume (slopes base**(h+1) times position)."""
    slopes = SLOPE_BASE ** np.arange(1, H + 1, dtype=np.float32)
    pos = np.arange(S, dtype=np.float32)
    exp = np.broadcast_to(
        (slopes[:, None] * pos[None, :])[None], (B, H, S)
    ).reshape(B * H, 1, S)
    return np.allclose(alibi, exp, rtol=1e-4, atol=1e-3)


def _classify_mask(mask):
    """mask: [B, 1, S, S] float32 -> 'none' | 'causal' | 'data'."""
    if not np.any(mask):
        return "none"
    tril = np.tril(np.ones((S, S), dtype=bool))
    for b in range(mask.shape[0]):
        m = mask[b, 0]
        if not (np.all(m[tril] == 0.0) and np.all(m[~tril] <= -1.0e8)):
            return "data"
    return "causal"


def kernel(
    hidden_states,
    residual,
    alibi,
    attention_mask,
    W_qkv,
    b_qkv,
    W_dense,
    b_dense,
):
    hidden_states = np.asarray(hidden_states, dtype=np.float32)
    residual = np.asarray(residual, dtype=np.float32)
    alibi = np.asarray(alibi, dtype=np.float32)
    attention_mask = np.asarray(attention_mask, dtype=np.float32)
    W_qkv = np.asarray(W_qkv, dtype=np.float32)
    b_qkv = np.asarray(b_qkv, dtype=np.float32)
    W_dense = np.asarray(W_dense, dtype=np.float32)
    b_dense = np.asarray(b_dense, dtype=np.float32)

    mode = _classify_mask(attention_mask)
    trim = bool(_bloom_alibi_ok(alibi))
    nc = _get_program(mode, trim)

    # W_qkv row blocks per head: rows h*384+[0:128) = q, +128 k, +256 v
    wq = W_qkv.reshape(H, 3, HD, D)[:, 0]  # [H, HD, D]
    wk = W_qkv.reshape(H, 3, HD, D)[:, 1]
    wv = W_qkv.reshape(H, 3, HD, D)[:, 2]
    bq = b_qkv.reshape(H, 3, HD)[:, 0]  # [H, HD]
    bk = b_qkv.reshape(H, 3, HD)[:, 1]
    bv = b_qkv.reshape(H, 3, HD)[:, 2]

    onesp1 = np.ones((128, 1), dtype=BF16_NP)
    ones1b = np.ones((1, 128), dtype=BF16_NP)
    shiftw_np = np.concatenate(
        [np.full((1, 128), SHIFT_ONES, np.float32), np.zeros((1, 128), np.float32)],
        axis=1,
    ).astype(F8_NP)

    xh_by_batch = []
    for b in range(B):
        xt = np.ascontiguousarray(hidden_states[b].T)
        xh_by_batch.append(xt.astype(F8_NP))

    def _hl_split(w):  # [D, F] fp32*WSCALE -> [D, 2F] fp8 hi|lo interleaved
        hi = w.astype(F8_NP)
        lo = (w - hi.astype(np.float32)).astype(F8_NP)
        return np.ascontiguousarray(
            np.stack([hi, lo], axis=1).reshape(w.shape[0], 2 * w.shape[1])
        )

    maskt_by_batch = None
    if mode == "data":
        # Clamp very-negative mask values: anything <= -190 already gives an
        # exact 0 after exp, and bounding |c| keeps the shift vector sane.
        # Clamp must exceed the full ALiBi span (~1450): a masked far key
        # has a[k] up to that much ABOVE the allowed maximum, and the clamp
        # has to keep a[k] + mask at least ~100 below c for an exact 0.
        attention_mask = np.maximum(attention_mask, np.float32(-1800.0))
        # the exp act multiplies the whole psum by INV_NORM, so pre-divide
        maskt_by_batch = [
            np.ascontiguousarray(attention_mask[b, 0].T / INV_NORM).astype(np.float32)
            for b in range(B)
        ]

    in_maps = []
    for c in range(NCORES):
        b = c // 4
        g = c % 4
        heads = [4 * j + g for j in range(HPC)]  # slot j -> head 4j+g

        wq_c = wq[heads].reshape(HPC * HD, D)  # [512, D]
        wk_c = wk[heads].reshape(HPC * HD, D)
        wv_c = wv[heads].reshape(HPC * HD, D)
        wd_c = W_dense[:, [h * HD + i for h in heads for i in range(HD)]]  # [D, 512]

        bqk_np = np.stack(
            [bq[h] for h in heads] + [bk[h] for h in heads], axis=1
        ).astype(np.float32)  # [128, 8]

        # per-head alibi columns [128, HPC*NKT] and shift c
        al = np.empty((128, HPC * NKT), dtype=np.float32)
        negc_np = np.empty((HPC, S), dtype=np.float32)
        fold = _shift_fold(mode, trim)
        for hl, h in enumerate(heads):
            a = alibi[b * H + h, 0]  # [S]
            if mode == "none":
                c_vec = np.full(S, a.max(), dtype=np.float32)
            elif mode == "causal":
                c_vec = np.maximum.accumulate(a)
            else:
                c_vec = (a[None, :] + attention_mask[b, 0]).max(axis=1)
            bias_cols = a.reshape(NKT, 128).T
            if fold[hl]:
                # constant shift folded into the fp32 act bias
                bias_cols = bias_cols - (c_vec.max() + c_vec.min()) / 2.0
                negc_np[hl] = 0.0
            else:
                negc_np[hl] = -c_vec / (INV_NORM * SHIFT_ONES)
            al[:, hl * NKT : (hl + 1) * NKT] = bias_cols

        wdt_c = np.ascontiguousarray(wd_c.T) * WSCALE  # [512, D]
        wdh_np = wdt_c.astype(F8_NP)
        wdl_np = (wdt_c - wdh_np.astype(np.float32)).astype(F8_NP)

        im = {
            "xh": xh_by_batch[b],
            "wqt": np.ascontiguousarray(wq_c.T * WSCALE).astype(F8_NP),
            "wkt": np.ascontiguousarray(wk_c.T * WSCALE).astype(F8_NP),
            "wvt": _hl_split(np.ascontiguousarray(wv_c.T) * WSCALE),
            "wdh": wdh_np,
            "wdl": wdl_np,
            "bqk": bqk_np,
            "alib": al,
            "negc": negc_np.reshape(1, HPC * S).astype(F8_NP),
            "shiftw": shiftw_np,
            "onesp1": onesp1,
            "ones1b": ones1b,
        }
        if mode == "data":
            im["maskt"] = maskt_by_batch[b]
        in_maps.append(im)

    res = None
    last_exc = None
    for attempt in range(3):
        try:
            res = bass_utils.run_bass_kernel_spmd(
                nc, in_maps, core_ids=list(range(NCORES))
            )
            break
        except Exception as e:  # transient device wedges (NRT_EXEC_*) happen
            last_exc = e
            time.sleep(2.0 * (attempt + 1))
    if res is None:
        raise last_exc

    # host-side v-bias fold: (ctx + bv) @ Wd^T = ctx @ Wd^T + bv @ Wd^T
    bv_flat = bv.reshape(D)
    bv_term = bv_flat @ W_dense.T  # [D]

    out = np.empty((B, S, D), dtype=np.float32)
    for b in range(B):
        acc = b_dense[None, :] + bv_term[None, :] + residual[b]
        for g in range(4):
            acc = acc + res.results[b * 4 + g]["outp"].astype(np.float32)
        out[b] = acc
    return out
